# revision 46
# baseline (speedup 1.0000x reference)
"""Trainium2 Bass kernel for nn_EquiStructureDecoder (8-core SPMD).

Key algebraic fact used: the network's outputs (coord_pred, attr_pred,
global_pred) depend only on the hidden stream h.  In each block,
h <- h + softmax(qk^T/sqrt(D)) @ v  uses only h; the coordinate stream x
(rel_x / edge_feat / coord_w / delta_x) never feeds back into h and is
not part of the returned outputs, so it is dead code and is not computed.
This is exact (dataflow equivalence of the h path), not an approximation.

Distribution (row-parallel over queries, per the sharding hint):
  - each of the 8 cores owns a 128-row slab of h (carried transposed,
    hT[d, i'], fp32 residual; bf16 operands for matmuls)
  - k/v are computed from the full (replicated) h each layer
  - after blocks 1 and 2 the updated transposed slabs are AllGather'd
    (bf16); the gather output is directly the stacked hT blocks
  - the global head needs only segment-sums of h, which are linear:
    each core computes its partial [D, G]; one 4KB AllGather of the
    partials + a local tree-sum replaces gathering all of h
  - softmax is computed without max-subtraction (scores for this model
    are O(0.1); exp is safe in fp32) and rows are normalized at the
    residual update via an outer-product broadcast of 1/rowsum
"""

import sys

for _p in ("/opt/trn_rl_repo",):
    if _p not in sys.path:
        sys.path.insert(0, _p)

import numpy as np
import ml_dtypes

import concourse.bass as bass
import concourse.bacc as bacc
import concourse.tile as tile
from concourse import mybir
from concourse import bass_utils

N = 1024
D = 128
NC = 8
S = N // NC        # 128 rows per core
L = 3
NG = 8
A = 16
G = 8
INV_SQRT_D = float(1.0 / np.sqrt(np.float32(D)))

F32 = mybir.dt.float32
BF16 = mybir.dt.bfloat16
AF = mybir.ActivationFunctionType
ALU = mybir.AluOpType

_BF = ml_dtypes.bfloat16


def _ts(i, size=128):
    return slice(i * size, (i + 1) * size)


class _Blob:
    """Column-packed constant blob: host array + SBUF slice bookkeeping."""

    def __init__(self, parts, dtype):
        self.parts = parts
        self.dtype = dtype
        self.cols = 0
        self.sections = {}
        self.arrays = []

    def add(self, name, arr):
        arr = np.asarray(arr)
        rows, cols = arr.shape
        assert rows <= self.parts
        self.sections[name] = (self.cols, cols, rows)
        self.cols += cols
        self.arrays.append(arr)
        return name

    def build(self):
        out = np.zeros((self.parts, self.cols), dtype=self.dtype)
        for (name, (off, cols, rows)), arr in zip(
            self.sections.items(), self.arrays
        ):
            out[:rows, off : off + cols] = arr
        return out


# blob section layouts (host + device must agree); filled in _build_in_maps
_B32 = None   # [128, *] f32, per-core
_BBF = None   # [128, *] bf16, replicated
_S32 = None   # [16, *] f32, per-core
_SBF = None   # [16, *] bf16, replicated


def _make_blob_layouts():
    """Define blob column layouts with dummy arrays (shapes only)."""
    b32 = _Blob(128, np.float32)
    b32.add("condTpb_own", np.zeros((D, S)))
    b32.add("i128", np.zeros((D, D)))
    b32.add("cow", np.zeros((D, 2)))
    b32.add("cob", np.zeros((S, 2)))
    b32.add("aow", np.zeros((D, A)))
    b32.add("aob", np.zeros((S, A)))
    b32.add("gw1", np.zeros((D, D)))
    b32.add("gw2", np.zeros((D, G)))
    b32.add("gb1", np.zeros((NG, D)))
    b32.add("gb2", np.zeros((NG, G)))
    b32.add("bq", np.zeros((D, L)))
    b32.add("bk", np.zeros((D, L)))
    b32.add("bv", np.zeros((D, L)))
    b32.add("meanMT_own", np.zeros((S, NG)))
    b32.add("sel64", np.zeros((NC * NG, NG)))
    b32.add("gpart_bv", np.zeros((NG, D)))
    b32.add("gb1c", np.zeros((D, 1)))

    bbf = _Blob(128, _BF)
    bbf.add("wq", np.zeros((D, L * D)))
    bbf.add("wkT", np.zeros((D, L * D)))
    bbf.add("wv", np.zeros((D, L * D)))
    bbf.add("condTpb", np.zeros((D, N)))
    bbf.add("i128b", np.zeros((D, D)))
    bbf.add("bkb", np.zeros((D, L)))

    s32 = _Blob(16, np.float32)
    s32.add("coordT_own", np.zeros((2, S)))
    s32.add("attrT_own", np.zeros((A, S)))
    s32.add("tT_own", np.zeros((1, S)))
    s32.add("cw", np.zeros((2, D)))
    s32.add("aw", np.zeros((A, D)))
    s32.add("tw", np.zeros((1, D)))
    s32.add("ones", np.zeros((1, D)))

    sbf = _Blob(16, _BF)
    sbf.add("coordT", np.zeros((2, N)))
    sbf.add("attrT", np.zeros((A, N)))
    sbf.add("tT", np.zeros((1, N)))
    sbf.add("cwb", np.zeros((2, D)))
    sbf.add("awb", np.zeros((A, D)))
    sbf.add("twb", np.zeros((1, D)))
    return b32, bbf, s32, sbf


_LAYOUT = _make_blob_layouts()


def _build_program():
    b32l, bbfl, s32l, sbfl = _LAYOUT
    nc = bacc.Bacc(
        "TRN2",
        target_bir_lowering=False,
        debug=False,
        enable_asserts=False,
        num_devices=NC,
    )

    blob32 = nc.dram_tensor("blob32", [128, b32l.cols], F32, kind="ExternalInput").ap()
    blobbf = nc.dram_tensor("blobbf", [128, bbfl.cols], BF16, kind="ExternalInput").ap()
    sm32 = nc.dram_tensor("sm32", [16, s32l.cols], F32, kind="ExternalInput").ap()
    smbf = nc.dram_tensor("smbf", [16, sbfl.cols], BF16, kind="ExternalInput").ap()

    o_coord = nc.dram_tensor("o_coord", [S, 2], F32, kind="ExternalOutput").ap()
    o_attr = nc.dram_tensor("o_attr", [S, A], F32, kind="ExternalOutput").ap()
    o_global = nc.dram_tensor("o_global", [NG, G], F32, kind="ExternalOutput").ap()

    with tile.TileContext(nc) as tc:
        with (
            tc.tile_pool(name="const", bufs=1) as cp,
            tc.tile_pool(name="work", bufs=2) as wp,
            tc.tile_pool(name="psum", bufs=1, space="PSUM") as pp,
            tc.tile_pool(name="dram", bufs=1, space="DRAM") as dp,
        ):
            # ---------- constant blobs: 4 DMAs on 2 HWDGE queues ----------
            t32 = cp.tile([128, b32l.cols], F32, name="t32")
            tbf = cp.tile([128, bbfl.cols], BF16, name="tbf")
            u32 = cp.tile([16, s32l.cols], F32, name="u32")
            ubf = cp.tile([16, sbfl.cols], BF16, name="ubf")
            nc.scalar.dma_start(u32[:], sm32)
            nc.sync.dma_start(ubf[:], smbf)
            nc.scalar.dma_start(t32[:], blob32)
            nc.sync.dma_start(tbf[:], blobbf)

            def c32(name):
                off, cols, rows = b32l.sections[name]
                return t32[:rows, off : off + cols]

            def cbf(name):
                off, cols, rows = bbfl.sections[name]
                return tbf[:rows, off : off + cols]

            def c16(name):
                off, cols, rows = s32l.sections[name]
                return u32[:rows, off : off + cols]

            def c16b(name):
                off, cols, rows = sbfl.sections[name]
                return ubf[:rows, off : off + cols]

            def wsl(name, l):
                off, _, _ = bbfl.sections[name]
                return tbf[:, off + l * D : off + (l + 1) * D]

            def bsl(name, l):
                off, _, _ = b32l.sections[name]
                return t32[:, off + l : off + l + 1]

            def wbsl(name, l):
                off, _, _ = bbfl.sections[name]
                return tbf[:, off + l : off + l + 1]

            # ---------- h0 ----------
            # own slab fp32 first (exact residual carry; longest chain:
            # h_ownT -> bf16 -> qT -> m -> S)
            h_ownT = cp.tile([D, S], F32, name="h_ownT")
            ps0 = pp.tile([D, S], F32, tag="small", bufs=3, name="ps0")
            nc.tensor.matmul(ps0[:], c16("cw"), c16("coordT_own"), start=True, stop=False)
            nc.tensor.matmul(ps0[:], c16("aw"), c16("attrT_own"), start=False, stop=False)
            nc.tensor.matmul(ps0[:], c16("tw"), c16("tT_own"), start=False, stop=True)
            nc.vector.tensor_tensor(h_ownT[:], ps0[:], c32("condTpb_own"), op=ALU.add)
            h_ownT_b = cp.tile([D, S], BF16, name="h_ownT_b0")
            nc.scalar.activation(h_ownT_b[:], h_ownT[:], AF.Copy)

            # full h0 (bf16) straight into the hT layout used by k/v,
            # split into four quarter tiles for finer downstream deps
            hT_q = [
                cp.tile([D, 256], BF16, name=f"hT_q{q}_a") for q in range(4)
            ]
            for q in range(4):
                psf = pp.tile([D, 256], F32, tag="big", bufs=2, name="psf")
                sl = slice(q * 256, (q + 1) * 256)
                nc.tensor.matmul(psf[:], c16b("cwb"), c16b("coordT")[:, sl], start=True, stop=False)
                nc.tensor.matmul(psf[:], c16b("awb"), c16b("attrT")[:, sl], start=False, stop=False)
                nc.tensor.matmul(psf[:], c16b("twb"), c16b("tT")[:, sl], start=False, stop=True)
                nc.vector.tensor_tensor(hT_q[q][:], psf[:], cbf("condTpb")[:, sl], op=ALU.add)

            # ---------- attention blocks ----------
            for l in range(L):
                last = l == L - 1

                if l > 0:
                    # rebuild hT quarters from the AllGather output (stacked
                    # transposed blocks); first quarters on the fast HWDGE
                    # queues so S can start as soon as blocks 0-1 land
                    hT_q = [
                        wp.tile([D, 256], BF16, name=f"hT_q{q}")
                        for q in range(4)
                    ]
                    engs = [nc.sync, nc.scalar, nc.sync, nc.scalar,
                            nc.gpsimd, nc.gpsimd, nc.sync, nc.scalar]
                    for b in range(NC):
                        engs[b].dma_start(
                            hT_q[b // 2][:, _ts(b % 2)], ag_out[_ts(b), :]
                        )

                # q-side (all local; runs during the AllGather):
                #   qT = Wq^T h_ownT (+bq);  m = Wk qT;  beta = qT^T bk
                # so that S = m^T @ hT + beta (bk folded into exp bias)
                ps_q = pp.tile([D, S], F32, tag="small", bufs=3, name="ps_q")
                nc.tensor.matmul(ps_q[:], wsl("wq", l), h_ownT_b[:], start=True, stop=True)
                qT_b = wp.tile([D, S], BF16, name="qT_b")
                nc.scalar.activation(qT_b[:], ps_q[:], AF.Identity, bias=bsl("bq", l))
                ps_m = pp.tile([D, S], F32, tag="small", bufs=3, name="ps_m")
                nc.tensor.matmul(ps_m[:], wsl("wkT", l), qT_b[:], start=True, stop=True)
                m_b = wp.tile([D, S], BF16, name="m_b")
                nc.scalar.activation(m_b[:], ps_m[:], AF.Copy)
                ps_be = pp.tile([S, 1], F32, tag="small", bufs=3, name="ps_be")
                nc.tensor.matmul(ps_be[:], qT_b[:], wbsl("bkb", l), start=True, stop=True)
                beta_s = wp.tile([S, 1], F32, name="beta_s")
                nc.vector.tensor_scalar_mul(beta_s[:], ps_be[:], INV_SQRT_D)

                if last:
                    # global-head partial, part 1 (off the critical chain):
                    # partial(h2) accumulates into ps_g during this layer;
                    # partial(delta3) is added from agg_n after the update
                    ps_h2u = pp.tile([S, D], F32, tag="small", bufs=3, name="ps_h2u")
                    nc.tensor.transpose(ps_h2u[:], h_ownT[:], c32("i128"))
                    h2_u = wp.tile([S, D], F32, name="h2_u")
                    nc.scalar.activation(h2_u[:], ps_h2u[:], AF.Copy)
                    ps_g = pp.tile([NG, D], F32, tag="small", bufs=3, name="ps_g")
                    nc.tensor.matmul(ps_g[:], c32("meanMT_own"), h2_u[:], start=True, stop=True)

                # pipelined by j-quarters: S -> exp -> transpose -> ET -> agg
                ps_s = pp.tile([S, N], F32, tag="big", bufs=2, name="ps_s")
                ps_v = pp.tile([D, N], F32, tag="big", bufs=2, name="ps_v")
                v_b = wp.tile([D, N], BF16, name="v_b")
                e_b = wp.tile([S, N], BF16, name="e_b")
                rs2 = wp.tile([S, 2], F32, name="rs2")
                ps_et = pp.tile([S, N], BF16, tag="bigbf", bufs=1, name="ps_et")
                et_b = wp.tile([S, N], BF16, name="et_b")
                ps_a = pp.tile([S, D], F32, tag="small", bufs=3, name="ps_a")
                rowsum = wp.tile([S, 1], F32, name="rowsum")
                recip = wp.tile([S, 1], F32, name="recip")

                # S and v per quarter-tile (start as soon as each lands)
                for q in range(4):
                    sl = slice(q * 256, (q + 1) * 256)
                    nc.tensor.matmul(ps_s[:, sl], m_b[:], hT_q[q][:], start=True, stop=True)
                    for b in (2 * q, 2 * q + 1):
                        nc.tensor.matmul(ps_v[:, _ts(b)], hT_q[q][:, _ts(b % 2)], wsl("wv", l), start=True, stop=True)
                # E = exp(S/sqrt(D) + beta) unnormalized + half-rowsums
                for c in range(2):
                    hl = slice(c * 512, (c + 1) * 512)
                    nc.scalar.activation(
                        e_b[:, hl], ps_s[:, hl], AF.Exp, scale=INV_SQRT_D,
                        bias=beta_s[:], accum_out=rs2[:, c : c + 1],
                    )
                for c in range(2):
                    hl = slice(c * 512, (c + 1) * 512)
                    nc.vector.tensor_copy(v_b[:, hl], ps_v[:, hl])
                # transpose unnormalized E blocks (dense PE queue)
                for b in range(NC):
                    nc.tensor.transpose(ps_et[:, _ts(b)], e_b[:, _ts(b)], cbf("i128b"))
                nc.vector.tensor_copy(et_b[:, : 4 * S], ps_et[:, : 4 * S])
                nc.scalar.activation(et_b[:, 4 * S :], ps_et[:, 4 * S :], AF.Copy)
                # agg[i',d] += sum_b ET_b^T @ v_b  == E @ v  (untransposed
                # so 1/rowsum applies as a per-partition scalar)
                for b in range(NC):
                    nc.tensor.matmul(
                        ps_a[:], et_b[:, _ts(b)], v_b[:, _ts(b)],
                        start=(b == 0), stop=(b == NC - 1),
                        skip_group_check=True,
                    )

                # normalize rows, transpose back, and update the residual
                nc.vector.tensor_reduce(rowsum[:], rs2[:], axis=mybir.AxisListType.X, op=ALU.add)
                nc.vector.reciprocal(recip[:], rowsum[:])
                agg_n = wp.tile([S, D], F32, name="agg_n")
                nc.vector.tensor_scalar_mul(agg_n[:], ps_a[:], recip[:])
                if last:
                    # global-head partial, part 2: + meanM @ delta3 (the bv
                    # term is a host-computed constant added at the copy)
                    nc.tensor.matmul(
                        ps_g[:], c32("meanMT_own"), agg_n[:],
                        start=False, stop=True, skip_group_check=True,
                    )
                    pg_s = wp.tile([NG, D], F32, name="pg_s")
                    nc.vector.tensor_tensor(pg_s[:], ps_g[:], c32("gpart_bv"), op=ALU.add)
                    ar_in = dp.tile([NG, D], F32, name="ar_in")
                    ar_out = dp.tile([NC * NG, D], F32, name="ar_out", addr_space="Shared")
                    nc.gpsimd.dma_start(ar_in[:], pg_s[:])
                    nc.gpsimd.collective_compute(
                        "AllGather",
                        ALU.bypass,
                        replica_groups=[list(range(NC))],
                        ins=[ar_in[:]],
                        outs=[ar_out[:]],
                    )
                ps_at2 = pp.tile([D, S], F32, tag="small", bufs=3, name="ps_at2")
                nc.tensor.transpose(ps_at2[:], agg_n[:], c32("i128"))

                # h <- h + agg^T + bv; bf16 copy first (feeds q + AllGather);
                # for the last layer only the fp32 update matters
                if not last:
                    h_ownT_b = wp.tile([D, S], BF16, name="h_ownT_b")
                    nc.vector.scalar_tensor_tensor(
                        h_ownT_b[:], ps_at2[:], bsl("bv", l), h_ownT[:], op0=ALU.add, op1=ALU.add
                    )
                    ag_in = dp.tile([D, S], BF16, name=f"ag_in{l}")
                    ag_out = dp.tile([N, S], BF16, name=f"ag_out{l}", addr_space="Shared")
                    nc.gpsimd.dma_start(ag_in[:], h_ownT_b[:])
                    nc.gpsimd.collective_compute(
                        "AllGather",
                        ALU.bypass,
                        replica_groups=[list(range(NC))],
                        ins=[ag_in[:]],
                        outs=[ag_out[:]],
                    )
                h_new = cp.tile([D, S], F32, name=f"h_new{l}")
                nc.vector.scalar_tensor_tensor(
                    h_new[:], ps_at2[:], bsl("bv", l), h_ownT[:], op0=ALU.add, op1=ALU.add
                )
                h_ownT = h_new

            # ---------- coord/attr heads (overlap the AllGather) ----------
            ps_c = pp.tile([S, 2], F32, tag="small", bufs=3, name="ps_c")
            nc.tensor.matmul(ps_c[:], h_ownT[:], c32("cow"), start=True, stop=True)
            oc_s = wp.tile([S, 2], F32, name="oc_s")
            nc.vector.tensor_tensor(oc_s[:], ps_c[:], c32("cob"), op=ALU.add)
            nc.scalar.dma_start(o_coord, oc_s[:])

            ps_at = pp.tile([S, A], F32, tag="small", bufs=3, name="ps_at")
            nc.tensor.matmul(ps_at[:], h_ownT[:], c32("aow"), start=True, stop=True)
            oa_s = wp.tile([S, A], F32, name="oa_s")
            nc.vector.tensor_tensor(oa_s[:], ps_at[:], c32("aob"), op=ALU.add)
            nc.scalar.dma_start(o_attr, oa_s[:])

            # ---------- global MLP ----------
            # one DMA for all gathered partials; reduce over cores on the PE:
            # hgT[d, g] = parts^T @ sel64
            parts = wp.tile([NC * NG, D], F32, name="parts")
            nc.sync.dma_start(parts[:], ar_out[:])
            ps_hg = pp.tile([D, NG], F32, tag="small", bufs=3, name="ps_hg")
            nc.tensor.matmul(ps_hg[:], parts[:], c32("sel64"), start=True, stop=True)
            hgT = wp.tile([D, NG], F32, name="hgT")
            nc.vector.tensor_copy(hgT[:], ps_hg[:])
            # z1 computed transposed: z1T[e,g] = gW1^T @ h_graphT, so gb1
            # is a per-partition bias fused into one Relu activation
            ps_z1 = pp.tile([D, NG], F32, tag="small", bufs=3, name="ps_z1")
            nc.tensor.matmul(ps_z1[:], c32("gw1"), hgT[:], start=True, stop=True)
            z1t = wp.tile([D, NG], F32, name="z1t")
            nc.scalar.activation(z1t[:], ps_z1[:], AF.Relu, bias=c32("gb1c"))
            ps_z2 = pp.tile([NG, G], F32, tag="small", bufs=3, name="ps_z2")
            nc.tensor.matmul(ps_z2[:], z1t[:], c32("gw2"), start=True, stop=True)
            og_s = wp.tile([NG, G], F32, name="og_s")
            nc.vector.tensor_tensor(og_s[:], ps_z2[:], c32("gb2"), op=ALU.add)
            nc.sync.dma_start(o_global, og_s[:])

    nc.compile()
    return nc


_PROGRAM = None


def _get_program():
    global _PROGRAM
    if _PROGRAM is None:
        _PROGRAM = _build_program()
    return _PROGRAM


def _build_in_maps(inputs):
    f32 = np.float32
    coord = np.asarray(inputs["theta_coord"], f32)        # [N, 2]
    attr = np.asarray(inputs["theta_attr"], f32)          # [N, A]
    t = np.asarray(inputs["t"], f32)                      # [N, 1]
    cond = np.asarray(inputs["condition"], f32)           # [N, D]
    batch = np.asarray(inputs["batch"]).astype(np.int64)  # [N]

    cw = np.asarray(inputs["coord_enc_W"], f32)
    aw = np.asarray(inputs["attr_enc_W"], f32)
    tw = np.asarray(inputs["time_W"], f32)
    enc_b = (
        np.asarray(inputs["coord_enc_b"], f32)
        + np.asarray(inputs["attr_enc_b"], f32)
        + np.asarray(inputs["time_b"], f32)
    )
    condTpb = cond.T + enc_b[:, None]                     # [D, N]

    wq = np.asarray(inputs["Wq"], f32)                    # [L, D, D]
    wk = np.asarray(inputs["Wk"], f32)
    wv = np.asarray(inputs["Wv"], f32)
    bq = np.asarray(inputs["bq"], f32)                    # [L, D]
    bk = np.asarray(inputs["bk"], f32)
    bv = np.asarray(inputs["bv"], f32)

    counts = np.bincount(batch, minlength=NG).astype(f32)
    onehot = (batch[None, :] == np.arange(NG)[:, None]).astype(f32)
    meanM = onehot / np.maximum(counts, 1.0)[:, None]     # [G, N]

    b32l, bbfl, s32l, sbfl = _LAYOUT

    # replicated bf16 blob
    bbf = _Blob(128, _BF)
    bbf.add("wq", np.concatenate([wq[l] for l in range(L)], 1).astype(_BF))
    bbf.add("wkT", np.concatenate([wk[l].T for l in range(L)], 1).astype(_BF))
    bbf.add("wv", np.concatenate([wv[l] for l in range(L)], 1).astype(_BF))
    bbf.add("condTpb", condTpb.astype(_BF))
    bbf.add("i128b", np.eye(D, dtype=f32).astype(_BF))
    bbf.add("bkb", bk.T.astype(_BF))
    blobbf = bbf.build()

    sbf = _Blob(16, _BF)
    sbf.add("coordT", coord.T.astype(_BF))
    sbf.add("attrT", attr.T.astype(_BF))
    sbf.add("tT", t.T.astype(_BF))
    sbf.add("cwb", cw.astype(_BF))
    sbf.add("awb", aw.astype(_BF))
    sbf.add("twb", tw.astype(_BF))
    smbf = sbf.build()

    in_maps = []
    for c in range(NC):
        sl = slice(c * S, (c + 1) * S)

        b32 = _Blob(128, f32)
        b32.add("condTpb_own", condTpb[:, sl])
        b32.add("i128", np.eye(D, dtype=f32))
        b32.add("cow", np.asarray(inputs["coord_out_W"], f32))
        b32.add("cob", np.broadcast_to(np.asarray(inputs["coord_out_b"], f32), (S, 2)))
        b32.add("aow", np.asarray(inputs["attr_out_W"], f32))
        b32.add("aob", np.broadcast_to(np.asarray(inputs["attr_out_b"], f32), (S, A)))
        b32.add("gw1", np.asarray(inputs["gW1"], f32))
        b32.add("gw2", np.asarray(inputs["gW2"], f32))
        b32.add("gb1", np.broadcast_to(np.asarray(inputs["gb1"], f32), (NG, D)))
        b32.add("gb2", np.broadcast_to(np.asarray(inputs["gb2"], f32), (NG, G)))
        b32.add("bq", bq.T)
        b32.add("bk", bk.T)
        b32.add("bv", bv.T)
        b32.add("meanMT_own", meanM[:, sl].T)
        b32.add("sel64", np.tile(np.eye(NG, dtype=f32), (NC, 1)))
        # segment-sum contribution of the layer-3 v-bias, computed on host:
        # sum_{i' in slab} meanM[g,i'] * bv3[d]
        rowfrac = meanM[:, sl].sum(axis=1)                # [G]
        b32.add("gpart_bv", np.outer(rowfrac, bv[L - 1]))
        b32.add("gb1c", np.asarray(inputs["gb1"], f32).reshape(D, 1))
        blob32 = b32.build()

        s32 = _Blob(16, f32)
        s32.add("coordT_own", coord[sl].T)
        s32.add("attrT_own", attr[sl].T)
        s32.add("tT_own", t[sl].T)
        s32.add("cw", cw)
        s32.add("aw", aw)
        s32.add("tw", tw)
        s32.add("ones", np.ones((1, D), f32))
        sm32 = s32.build()

        in_maps.append(
            {"blob32": blob32, "blobbf": blobbf, "sm32": sm32, "smbf": smbf}
        )
    return in_maps


def kernel(**inputs):
    nc = _get_program()
    in_maps = _build_in_maps(inputs)
    res = bass_utils.run_bass_kernel_spmd(nc, in_maps, core_ids=list(range(NC)))
    coord_pred = np.concatenate(
        [res.results[c]["o_coord"] for c in range(NC)], axis=0
    )
    attr_pred = np.concatenate(
        [res.results[c]["o_attr"] for c in range(NC)], axis=0
    )
    global_pred = res.results[0]["o_global"]
    return (
        coord_pred.astype(np.float32),
        attr_pred.astype(np.float32),
        global_pred.astype(np.float32),
    )


# revision 47
# speedup vs baseline: 1.0735x; 1.0735x over previous
"""Trainium2 Bass kernel for nn_EquiStructureDecoder (8-core SPMD).

Key algebraic fact used: the network's outputs (coord_pred, attr_pred,
global_pred) depend only on the hidden stream h.  In each block,
h <- h + softmax(qk^T/sqrt(D)) @ v  uses only h; the coordinate stream x
(rel_x / edge_feat / coord_w / delta_x) never feeds back into h and is
not part of the returned outputs, so it is dead code and is not computed.
This is exact (dataflow equivalence of the h path), not an approximation.

Distribution (row-parallel over queries, per the sharding hint):
  - each of the 8 cores owns a 128-row slab of h (carried transposed,
    hT[d, i'], fp32 residual; bf16 operands for matmuls)
  - k/v are computed from the full (replicated) h each layer
  - after blocks 1 and 2 the updated transposed slabs are AllGather'd
    (bf16); the gather output is directly the stacked hT blocks
  - the global head needs only segment-sums of h, which are linear:
    each core computes its partial [D, G]; one 4KB AllGather of the
    partials + a local tree-sum replaces gathering all of h
  - softmax is computed without max-subtraction (scores for this model
    are O(0.1); exp is safe in fp32) and rows are normalized at the
    residual update via an outer-product broadcast of 1/rowsum
"""

import sys

for _p in ("/opt/trn_rl_repo",):
    if _p not in sys.path:
        sys.path.insert(0, _p)

import numpy as np
import ml_dtypes

import concourse.bass as bass
import concourse.bacc as bacc
import concourse.tile as tile
from concourse import mybir
from concourse import bass_utils

N = 1024
D = 128
NC = 8
S = N // NC        # 128 rows per core
L = 3
NG = 8
A = 16
G = 8
INV_SQRT_D = float(1.0 / np.sqrt(np.float32(D)))

F32 = mybir.dt.float32
BF16 = mybir.dt.bfloat16
AF = mybir.ActivationFunctionType
ALU = mybir.AluOpType

_BF = ml_dtypes.bfloat16


def _ts(i, size=128):
    return slice(i * size, (i + 1) * size)


class _Blob:
    """Column-packed constant blob: host array + SBUF slice bookkeeping."""

    def __init__(self, parts, dtype):
        self.parts = parts
        self.dtype = dtype
        self.cols = 0
        self.sections = {}
        self.arrays = []

    def add(self, name, arr):
        arr = np.asarray(arr)
        rows, cols = arr.shape
        assert rows <= self.parts
        self.sections[name] = (self.cols, cols, rows)
        self.cols += cols
        self.arrays.append(arr)
        return name

    def build(self):
        out = np.zeros((self.parts, self.cols), dtype=self.dtype)
        for (name, (off, cols, rows)), arr in zip(
            self.sections.items(), self.arrays
        ):
            out[:rows, off : off + cols] = arr
        return out


# blob section layouts (host + device must agree); filled in _build_in_maps
_B32 = None   # [128, *] f32, per-core
_BBF = None   # [128, *] bf16, replicated
_S32 = None   # [16, *] f32, per-core
_SBF = None   # [16, *] bf16, replicated


def _make_blob_layouts():
    """Define blob column layouts with dummy arrays (shapes only)."""
    b32 = _Blob(128, np.float32)
    b32.add("condTpb_own", np.zeros((D, S)))
    b32.add("i128", np.zeros((D, D)))
    b32.add("cow", np.zeros((D, 2)))
    b32.add("cob", np.zeros((S, 2)))
    b32.add("aow", np.zeros((D, A)))
    b32.add("aob", np.zeros((S, A)))
    b32.add("gw1", np.zeros((D, D)))
    b32.add("gw2", np.zeros((D, G)))
    b32.add("gb1", np.zeros((NG, D)))
    b32.add("gb2", np.zeros((NG, G)))
    b32.add("bq", np.zeros((D, L)))
    b32.add("bk", np.zeros((D, L)))
    b32.add("bv", np.zeros((D, L)))
    b32.add("meanMT_own", np.zeros((S, NG)))
    b32.add("sel64", np.zeros((NC * NG, NG)))
    b32.add("gpart_bv", np.zeros((NG, D)))
    b32.add("gb1c", np.zeros((D, 1)))

    bbf = _Blob(128, _BF)
    bbf.add("wq", np.zeros((D, L * D)))
    bbf.add("wkT", np.zeros((D, L * D)))
    bbf.add("wv", np.zeros((D, L * D)))
    bbf.add("condTpb", np.zeros((D, N)))
    bbf.add("i128b", np.zeros((D, D)))
    bbf.add("bkb", np.zeros((D, L)))

    s32 = _Blob(16, np.float32)
    s32.add("coordT_own", np.zeros((2, S)))
    s32.add("attrT_own", np.zeros((A, S)))
    s32.add("tT_own", np.zeros((1, S)))
    s32.add("cw", np.zeros((2, D)))
    s32.add("aw", np.zeros((A, D)))
    s32.add("tw", np.zeros((1, D)))
    s32.add("ones", np.zeros((1, D)))

    sbf = _Blob(16, _BF)
    sbf.add("coordT", np.zeros((2, N)))
    sbf.add("attrT", np.zeros((A, N)))
    sbf.add("tT", np.zeros((1, N)))
    sbf.add("cwb", np.zeros((2, D)))
    sbf.add("awb", np.zeros((A, D)))
    sbf.add("twb", np.zeros((1, D)))
    return b32, bbf, s32, sbf


_LAYOUT = _make_blob_layouts()


def _build_program():
    b32l, bbfl, s32l, sbfl = _LAYOUT
    nc = bacc.Bacc(
        "TRN2",
        target_bir_lowering=False,
        debug=False,
        enable_asserts=False,
        num_devices=NC,
    )

    blob32 = nc.dram_tensor("blob32", [128, b32l.cols], F32, kind="ExternalInput").ap()
    blobbf = nc.dram_tensor("blobbf", [128, bbfl.cols], BF16, kind="ExternalInput").ap()
    sm32 = nc.dram_tensor("sm32", [16, s32l.cols], F32, kind="ExternalInput").ap()
    smbf = nc.dram_tensor("smbf", [16, sbfl.cols], BF16, kind="ExternalInput").ap()

    o_coord = nc.dram_tensor("o_coord", [S, 2], F32, kind="ExternalOutput").ap()
    o_attr = nc.dram_tensor("o_attr", [S, A], F32, kind="ExternalOutput").ap()
    o_global = nc.dram_tensor("o_global", [NG, G], F32, kind="ExternalOutput").ap()

    with tile.TileContext(nc) as tc:
        with (
            tc.tile_pool(name="const", bufs=1) as cp,
            tc.tile_pool(name="work", bufs=2) as wp,
            tc.tile_pool(name="psum", bufs=1, space="PSUM") as pp,
            tc.tile_pool(name="dram", bufs=1, space="DRAM") as dp,
        ):
            # ---------- constant blobs: 4 DMAs on 2 HWDGE queues ----------
            t32 = cp.tile([128, b32l.cols], F32, name="t32")
            tbf = cp.tile([128, bbfl.cols], BF16, name="tbf")
            u32 = cp.tile([16, s32l.cols], F32, name="u32")
            ubf = cp.tile([16, sbfl.cols], BF16, name="ubf")
            nc.scalar.dma_start(u32[:], sm32)
            nc.sync.dma_start(ubf[:], smbf)
            nc.scalar.dma_start(t32[:], blob32)
            nc.sync.dma_start(tbf[:], blobbf)

            def c32(name):
                off, cols, rows = b32l.sections[name]
                return t32[:rows, off : off + cols]

            def cbf(name):
                off, cols, rows = bbfl.sections[name]
                return tbf[:rows, off : off + cols]

            def c16(name):
                off, cols, rows = s32l.sections[name]
                return u32[:rows, off : off + cols]

            def c16b(name):
                off, cols, rows = sbfl.sections[name]
                return ubf[:rows, off : off + cols]

            def wsl(name, l):
                off, _, _ = bbfl.sections[name]
                return tbf[:, off + l * D : off + (l + 1) * D]

            def bsl(name, l):
                off, _, _ = b32l.sections[name]
                return t32[:, off + l : off + l + 1]

            def wbsl(name, l):
                off, _, _ = bbfl.sections[name]
                return tbf[:, off + l : off + l + 1]

            # ---------- h0 ----------
            # own slab fp32 first (exact residual carry; longest chain:
            # h_ownT -> bf16 -> qT -> m -> S)
            h_ownT = cp.tile([D, S], F32, name="h_ownT")
            ps0 = pp.tile([D, S], F32, tag="small", bufs=3, name="ps0")
            nc.tensor.matmul(ps0[:], c16("cw"), c16("coordT_own"), start=True, stop=False)
            nc.tensor.matmul(ps0[:], c16("aw"), c16("attrT_own"), start=False, stop=False)
            nc.tensor.matmul(ps0[:], c16("tw"), c16("tT_own"), start=False, stop=True)
            nc.vector.tensor_tensor(h_ownT[:], ps0[:], c32("condTpb_own"), op=ALU.add)
            h_ownT_b = cp.tile([D, S], BF16, name="h_ownT_b0")
            nc.scalar.activation(h_ownT_b[:], h_ownT[:], AF.Copy)

            # full h0 (bf16) straight into the hT layout used by k/v,
            # split into four quarter tiles for finer downstream deps
            hT_q = [
                cp.tile([D, 256], BF16, name=f"hT_q{q}_a") for q in range(4)
            ]
            for q in range(4):
                psf = pp.tile([D, 256], F32, tag="big", bufs=2, name="psf")
                sl = slice(q * 256, (q + 1) * 256)
                nc.tensor.matmul(psf[:], c16b("cwb"), c16b("coordT")[:, sl], start=True, stop=False)
                nc.tensor.matmul(psf[:], c16b("awb"), c16b("attrT")[:, sl], start=False, stop=False)
                nc.tensor.matmul(psf[:], c16b("twb"), c16b("tT")[:, sl], start=False, stop=True)
                nc.vector.tensor_tensor(hT_q[q][:], psf[:], cbf("condTpb")[:, sl], op=ALU.add)

            # ---------- attention blocks ----------
            for l in range(L):
                last = l == L - 1

                if l > 0:
                    # rebuild hT quarters from the AllGather output (stacked
                    # transposed blocks); first quarters on the fast HWDGE
                    # queues so S can start as soon as blocks 0-1 land
                    hT_q = [
                        wp.tile([D, 256], BF16, name=f"hT_q{q}")
                        for q in range(4)
                    ]
                    engs = [nc.sync, nc.scalar, nc.sync, nc.scalar,
                            nc.gpsimd, nc.gpsimd, nc.sync, nc.scalar]
                    for b in range(NC):
                        engs[b].dma_start(
                            hT_q[b // 2][:, _ts(b % 2)], ag_out[_ts(b), :]
                        )

                # q-side (all local; runs during the AllGather):
                #   qT = Wq^T h_ownT (+bq);  m = Wk qT;  beta = qT^T bk
                # so that S = m^T @ hT + beta (bk folded into exp bias)
                ps_q = pp.tile([D, S], F32, tag="small", bufs=3, name="ps_q")
                nc.tensor.matmul(ps_q[:], wsl("wq", l), h_ownT_b[:], start=True, stop=True)
                qT_b = wp.tile([D, S], BF16, name="qT_b")
                nc.scalar.activation(qT_b[:], ps_q[:], AF.Identity, bias=bsl("bq", l))
                ps_m = pp.tile([D, S], F32, tag="small", bufs=3, name="ps_m")
                nc.tensor.matmul(ps_m[:], wsl("wkT", l), qT_b[:], start=True, stop=True)
                m_b = wp.tile([D, S], BF16, name="m_b")
                nc.scalar.activation(m_b[:], ps_m[:], AF.Copy)
                ps_be = pp.tile([S, 1], F32, tag="small", bufs=3, name="ps_be")
                nc.tensor.matmul(ps_be[:], qT_b[:], wbsl("bkb", l), start=True, stop=True)
                beta_s = wp.tile([S, 1], F32, name="beta_s")
                nc.vector.tensor_scalar_mul(beta_s[:], ps_be[:], INV_SQRT_D)

                if last:
                    # global-head partial, part 1 (off the critical chain):
                    # partial(h2) accumulates into ps_g during this layer;
                    # partial(delta3) is added from agg_n after the update
                    ps_h2u = pp.tile([S, D], F32, tag="small", bufs=3, name="ps_h2u")
                    nc.tensor.transpose(ps_h2u[:], h_ownT[:], c32("i128"))
                    h2_u = wp.tile([S, D], F32, name="h2_u")
                    nc.scalar.activation(h2_u[:], ps_h2u[:], AF.Copy)
                    ps_g = pp.tile([NG, D], F32, tag="small", bufs=3, name="ps_g")
                    nc.tensor.matmul(ps_g[:], c32("meanMT_own"), h2_u[:], start=True, stop=True)

                # pipelined by j-quarters: S -> exp -> transpose -> ET -> agg
                ps_s = pp.tile([S, N], F32, tag="big", bufs=2, name="ps_s")
                ps_v = pp.tile([D, N], F32, tag="big", bufs=2, name="ps_v")
                v_b = wp.tile([D, N], BF16, name="v_b")
                e_b = wp.tile([S, N], BF16, name="e_b")
                rs2 = wp.tile([S, 2], F32, name="rs2")
                ps_et = pp.tile([S, N], BF16, tag="bigbf", bufs=1, name="ps_et")
                et_b = wp.tile([S, N], BF16, name="et_b")
                ps_a = pp.tile([S, D], F32, tag="small", bufs=3, name="ps_a")
                rowsum = wp.tile([S, 1], F32, name="rowsum")
                recip = wp.tile([S, 1], F32, name="recip")

                for c in range(2):
                    hl = slice(c * 512, (c + 1) * 512)
                    # S and v per quarter-tile (start as soon as each lands)
                    for q in (2 * c, 2 * c + 1):
                        sl = slice(q * 256, (q + 1) * 256)
                        nc.tensor.matmul(ps_s[:, sl], m_b[:], hT_q[q][:], start=True, stop=True)
                        for b in (2 * q, 2 * q + 1):
                            nc.tensor.matmul(ps_v[:, _ts(b)], hT_q[q][:, _ts(b % 2)], wsl("wv", l), start=True, stop=True)
                    # E = exp(S/sqrt(D) + beta) unnormalized + half-rowsum
                    nc.scalar.activation(
                        e_b[:, hl], ps_s[:, hl], AF.Exp, scale=INV_SQRT_D,
                        bias=beta_s[:], accum_out=rs2[:, c : c + 1],
                    )
                    nc.vector.tensor_copy(v_b[:, hl], ps_v[:, hl])
                    # transpose unnormalized E blocks of this half
                    for b in range(4 * c, 4 * (c + 1)):
                        nc.tensor.transpose(ps_et[:, _ts(b)], e_b[:, _ts(b)], cbf("i128b"))
                    if c == 0:
                        nc.vector.tensor_copy(et_b[:, hl], ps_et[:, hl])
                    else:
                        nc.scalar.activation(et_b[:, hl], ps_et[:, hl], AF.Copy)
                    # agg[i',d] += sum_b ET_b^T @ v_b  == E @ v  (untransposed
                    # so 1/rowsum applies as a per-partition scalar)
                    for b in range(4 * c, 4 * (c + 1)):
                        nc.tensor.matmul(
                            ps_a[:], et_b[:, _ts(b)], v_b[:, _ts(b)],
                            start=(b == 0), stop=(b % 4 == 3),
                            skip_group_check=True,
                        )

                # normalize rows, transpose back, and update the residual
                nc.vector.tensor_reduce(rowsum[:], rs2[:], axis=mybir.AxisListType.X, op=ALU.add)
                nc.vector.reciprocal(recip[:], rowsum[:])
                agg_n = wp.tile([S, D], F32, name="agg_n")
                nc.vector.tensor_scalar_mul(agg_n[:], ps_a[:], recip[:])
                if last:
                    # global-head partial, part 2: + meanM @ delta3 (the bv
                    # term is a host-computed constant added at the copy)
                    nc.tensor.matmul(
                        ps_g[:], c32("meanMT_own"), agg_n[:],
                        start=False, stop=True, skip_group_check=True,
                    )
                    pg_s = wp.tile([NG, D], F32, name="pg_s")
                    nc.vector.tensor_tensor(pg_s[:], ps_g[:], c32("gpart_bv"), op=ALU.add)
                    ar_in = dp.tile([NG, D], F32, name="ar_in")
                    ar_out = dp.tile([NC * NG, D], F32, name="ar_out", addr_space="Shared")
                    nc.gpsimd.dma_start(ar_in[:], pg_s[:])
                    nc.gpsimd.collective_compute(
                        "AllGather",
                        ALU.bypass,
                        replica_groups=[list(range(NC))],
                        ins=[ar_in[:]],
                        outs=[ar_out[:]],
                    )
                ps_at2 = pp.tile([D, S], F32, tag="small", bufs=3, name="ps_at2")
                nc.tensor.transpose(ps_at2[:], agg_n[:], c32("i128"))

                # h <- h + agg^T + bv; bf16 copy first (feeds q + AllGather);
                # for the last layer only the fp32 update matters
                if not last:
                    h_ownT_b = wp.tile([D, S], BF16, name="h_ownT_b")
                    nc.vector.scalar_tensor_tensor(
                        h_ownT_b[:], ps_at2[:], bsl("bv", l), h_ownT[:], op0=ALU.add, op1=ALU.add
                    )
                    ag_in = dp.tile([D, S], BF16, name=f"ag_in{l}")
                    ag_out = dp.tile([N, S], BF16, name=f"ag_out{l}", addr_space="Shared")
                    nc.gpsimd.dma_start(ag_in[:], h_ownT_b[:])
                    nc.gpsimd.collective_compute(
                        "AllGather",
                        ALU.bypass,
                        replica_groups=[list(range(NC))],
                        ins=[ag_in[:]],
                        outs=[ag_out[:]],
                    )
                h_new = cp.tile([D, S], F32, name=f"h_new{l}")
                nc.vector.scalar_tensor_tensor(
                    h_new[:], ps_at2[:], bsl("bv", l), h_ownT[:], op0=ALU.add, op1=ALU.add
                )
                h_ownT = h_new

            # ---------- coord/attr heads (overlap the AllGather) ----------
            ps_c = pp.tile([S, 2], F32, tag="small", bufs=3, name="ps_c")
            nc.tensor.matmul(ps_c[:], h_ownT[:], c32("cow"), start=True, stop=True)
            oc_s = wp.tile([S, 2], F32, name="oc_s")
            nc.vector.tensor_tensor(oc_s[:], ps_c[:], c32("cob"), op=ALU.add)
            nc.scalar.dma_start(o_coord, oc_s[:])

            ps_at = pp.tile([S, A], F32, tag="small", bufs=3, name="ps_at")
            nc.tensor.matmul(ps_at[:], h_ownT[:], c32("aow"), start=True, stop=True)
            oa_s = wp.tile([S, A], F32, name="oa_s")
            nc.vector.tensor_tensor(oa_s[:], ps_at[:], c32("aob"), op=ALU.add)
            nc.scalar.dma_start(o_attr, oa_s[:])

            # ---------- global MLP ----------
            # one DMA for all gathered partials; reduce over cores on the PE:
            # hgT[d, g] = parts^T @ sel64
            parts = wp.tile([NC * NG, D], F32, name="parts")
            nc.sync.dma_start(parts[:], ar_out[:])
            ps_hg = pp.tile([D, NG], F32, tag="small", bufs=3, name="ps_hg")
            nc.tensor.matmul(ps_hg[:], parts[:], c32("sel64"), start=True, stop=True)
            hgT = wp.tile([D, NG], F32, name="hgT")
            nc.vector.tensor_copy(hgT[:], ps_hg[:])
            # z1 computed transposed: z1T[e,g] = gW1^T @ h_graphT, so gb1
            # is a per-partition bias fused into one Relu activation
            ps_z1 = pp.tile([D, NG], F32, tag="small", bufs=3, name="ps_z1")
            nc.tensor.matmul(ps_z1[:], c32("gw1"), hgT[:], start=True, stop=True)
            z1t = wp.tile([D, NG], F32, name="z1t")
            nc.scalar.activation(z1t[:], ps_z1[:], AF.Relu, bias=c32("gb1c"))
            ps_z2 = pp.tile([NG, G], F32, tag="small", bufs=3, name="ps_z2")
            nc.tensor.matmul(ps_z2[:], z1t[:], c32("gw2"), start=True, stop=True)
            og_s = wp.tile([NG, G], F32, name="og_s")
            nc.vector.tensor_tensor(og_s[:], ps_z2[:], c32("gb2"), op=ALU.add)
            nc.sync.dma_start(o_global, og_s[:])

    nc.compile()
    return nc


_PROGRAM = None


def _get_program():
    global _PROGRAM
    if _PROGRAM is None:
        _PROGRAM = _build_program()
    return _PROGRAM


def _build_in_maps(inputs):
    f32 = np.float32
    coord = np.asarray(inputs["theta_coord"], f32)        # [N, 2]
    attr = np.asarray(inputs["theta_attr"], f32)          # [N, A]
    t = np.asarray(inputs["t"], f32)                      # [N, 1]
    cond = np.asarray(inputs["condition"], f32)           # [N, D]
    batch = np.asarray(inputs["batch"]).astype(np.int64)  # [N]

    cw = np.asarray(inputs["coord_enc_W"], f32)
    aw = np.asarray(inputs["attr_enc_W"], f32)
    tw = np.asarray(inputs["time_W"], f32)
    enc_b = (
        np.asarray(inputs["coord_enc_b"], f32)
        + np.asarray(inputs["attr_enc_b"], f32)
        + np.asarray(inputs["time_b"], f32)
    )
    condTpb = cond.T + enc_b[:, None]                     # [D, N]

    wq = np.asarray(inputs["Wq"], f32)                    # [L, D, D]
    wk = np.asarray(inputs["Wk"], f32)
    wv = np.asarray(inputs["Wv"], f32)
    bq = np.asarray(inputs["bq"], f32)                    # [L, D]
    bk = np.asarray(inputs["bk"], f32)
    bv = np.asarray(inputs["bv"], f32)

    counts = np.bincount(batch, minlength=NG).astype(f32)
    onehot = (batch[None, :] == np.arange(NG)[:, None]).astype(f32)
    meanM = onehot / np.maximum(counts, 1.0)[:, None]     # [G, N]

    b32l, bbfl, s32l, sbfl = _LAYOUT

    # replicated bf16 blob
    bbf = _Blob(128, _BF)
    bbf.add("wq", np.concatenate([wq[l] for l in range(L)], 1).astype(_BF))
    bbf.add("wkT", np.concatenate([wk[l].T for l in range(L)], 1).astype(_BF))
    bbf.add("wv", np.concatenate([wv[l] for l in range(L)], 1).astype(_BF))
    bbf.add("condTpb", condTpb.astype(_BF))
    bbf.add("i128b", np.eye(D, dtype=f32).astype(_BF))
    bbf.add("bkb", bk.T.astype(_BF))
    blobbf = bbf.build()

    sbf = _Blob(16, _BF)
    sbf.add("coordT", coord.T.astype(_BF))
    sbf.add("attrT", attr.T.astype(_BF))
    sbf.add("tT", t.T.astype(_BF))
    sbf.add("cwb", cw.astype(_BF))
    sbf.add("awb", aw.astype(_BF))
    sbf.add("twb", tw.astype(_BF))
    smbf = sbf.build()

    in_maps = []
    for c in range(NC):
        sl = slice(c * S, (c + 1) * S)

        b32 = _Blob(128, f32)
        b32.add("condTpb_own", condTpb[:, sl])
        b32.add("i128", np.eye(D, dtype=f32))
        b32.add("cow", np.asarray(inputs["coord_out_W"], f32))
        b32.add("cob", np.broadcast_to(np.asarray(inputs["coord_out_b"], f32), (S, 2)))
        b32.add("aow", np.asarray(inputs["attr_out_W"], f32))
        b32.add("aob", np.broadcast_to(np.asarray(inputs["attr_out_b"], f32), (S, A)))
        b32.add("gw1", np.asarray(inputs["gW1"], f32))
        b32.add("gw2", np.asarray(inputs["gW2"], f32))
        b32.add("gb1", np.broadcast_to(np.asarray(inputs["gb1"], f32), (NG, D)))
        b32.add("gb2", np.broadcast_to(np.asarray(inputs["gb2"], f32), (NG, G)))
        b32.add("bq", bq.T)
        b32.add("bk", bk.T)
        b32.add("bv", bv.T)
        b32.add("meanMT_own", meanM[:, sl].T)
        b32.add("sel64", np.tile(np.eye(NG, dtype=f32), (NC, 1)))
        # segment-sum contribution of the layer-3 v-bias, computed on host:
        # sum_{i' in slab} meanM[g,i'] * bv3[d]
        rowfrac = meanM[:, sl].sum(axis=1)                # [G]
        b32.add("gpart_bv", np.outer(rowfrac, bv[L - 1]))
        b32.add("gb1c", np.asarray(inputs["gb1"], f32).reshape(D, 1))
        blob32 = b32.build()

        s32 = _Blob(16, f32)
        s32.add("coordT_own", coord[sl].T)
        s32.add("attrT_own", attr[sl].T)
        s32.add("tT_own", t[sl].T)
        s32.add("cw", cw)
        s32.add("aw", aw)
        s32.add("tw", tw)
        s32.add("ones", np.ones((1, D), f32))
        sm32 = s32.build()

        in_maps.append(
            {"blob32": blob32, "blobbf": blobbf, "sm32": sm32, "smbf": smbf}
        )
    return in_maps


def kernel(**inputs):
    nc = _get_program()
    in_maps = _build_in_maps(inputs)
    res = bass_utils.run_bass_kernel_spmd(nc, in_maps, core_ids=list(range(NC)))
    coord_pred = np.concatenate(
        [res.results[c]["o_coord"] for c in range(NC)], axis=0
    )
    attr_pred = np.concatenate(
        [res.results[c]["o_attr"] for c in range(NC)], axis=0
    )
    global_pred = res.results[0]["o_global"]
    return (
        coord_pred.astype(np.float32),
        attr_pred.astype(np.float32),
        global_pred.astype(np.float32),
    )
